# revision 1
# baseline (speedup 1.0000x reference)
"""Trainium2 Bass kernel for nn_Classifier (GNN edge classifier), v3.

Reference computation, per edge e with src s=idx[0,e], dst d=idx[1,e]:
    out[e] = W2 @ relu(W1 @ [x_disease[s]; x_drug[d]] + b1) + b2

Algebraic restructure: W1 = [W1a | W1b] (each [256,128]); fold |w2| into the
node tables and permute the hidden dim so the +/-1 signs of w2 are applied
by the first fold of a binary reduction tree:
    A = x_disease @ (|w2| . W1a).T + |w2| . b1     [n_nodes, 256]
    B = x_drug    @ (|w2| . W1b).T                 [n_nodes, 256]
    out[e] = sum_k sgn_k relu(A[s] + B[d])_[perm k] + b2   (b2 added on host)
Hidden columns are ordered [L | R] (128 each) with cross pairs (pos, neg)
first: t1[0:q] = relu_h(L) - relu_h(R); leftover same-sign pairs:
t1[q:128] = +-(relu_h(L) + relu_h(R)). Then t2 = t1[0:64] + t1[64:128] and a
free-axis add-reduce finish the dot product with sgn(w2).

Per-core plan (8-way data parallel over edges, ~125k edges/core):
  Phase B (Tile): load consts; compute A,B ([n_nodes,256] bf16) on device
    (PE matmuls) from host-transposed bf16 x tables; store to DRAM.
  Phase C per block of 4096 edges: 4+4 dma_gather (1024 idx each — the
    65-descriptor single-packet HW limit) from A/B over 4 SWDGE queues into
    [128, 32, 256] tiles (edge g*128+p at [p, g, :]); DVE flat add
    h = gA + gB; ACT relu in place; DVE signed tree (t1 sub/add, t2 add,
    reduce) -> r [128, 32] f32; 32x32 transposes; Sync DMA rows out.
"""

import sys
import types
from contextlib import ExitStack

import numpy as np

import concourse.bacc as bacc
import concourse.bass as bass
import concourse.mybir as mybir
import concourse.tile as tile

F32 = mybir.dt.float32
BF16 = mybir.dt.bfloat16
I16 = mybir.dt.int16


def _cdiv(a, b):
    return (a + b - 1) // b


class Cfg:
    def __init__(self, n_nodes=20000, e_core=125000, node_chunk=1024,
                 q=128, tail_neg=False):
        self.n_nodes = n_nodes
        self.e_core = e_core
        self.gi = 1024                       # idx per dma_gather (HW limit)
        self.gpb = 4                         # gathers per block per table
        self.epb = self.gi * self.gpb        # 4096 edges per block
        self.nblk = _cdiv(e_core, self.epb)
        self.e_pad = self.nblk * self.epb
        self.ngrp = self.epb // 128          # 32 groups per block
        self.idx_cols = self.e_pad // 16     # wrapped idx columns per table
        self.node_chunk = node_chunk
        self.n_rows = _cdiv(n_nodes, 128) * 128
        self.q = q                           # cross (pos,neg) pair count
        self.tail_neg = tail_neg             # leftover pairs are (neg,neg)


FULL_SHAPE = (20000, 125000)
N_CORES = 8
E_TOTAL = 1_000_000
NQ = 4  # SWDGE queues


def build(nc, io, cfg):
    """Emit the per-core program (Tile phases B + C)."""
    c = cfg
    stack = ExitStack()
    with stack:
        isrc_sb = stack.enter_context(
            nc.sbuf_tensor("isrc_sb", [128, c.idx_cols], I16))
        idst_sb = stack.enter_context(
            nc.sbuf_tensor("idst_sb", [128, c.idx_cols], I16))

        a_tab = nc.dram_tensor("a_tab", [c.n_rows, 256], BF16)
        b_tab = nc.dram_tensor("b_tab", [c.n_rows, 256], BF16)

        with tile.TileContext(nc) as tc:
            with tc.tile_pool(name="const", bufs=1) as cpool:
                w1at_b = cpool.tile([128, 256], BF16, tag="w1at_b")
                w1bt_b = cpool.tile([128, 256], BF16, tag="w1bt_b")
                for name, dst in (("w1at", w1at_b), ("w1bt", w1bt_b)):
                    f = cpool.tile([128, 256], F32, tag=name + "_f")
                    nc.sync.dma_start(f[:], io[name][:])
                    nc.vector.tensor_copy(dst[:], f[:])
                b1_sb = cpool.tile([128, 256], F32, tag="b1_sb")
                nc.sync.dma_start(b1_sb[:], io["b1bc"][:])
                nc.sync.dma_start(isrc_sb[:], io["isrc"][:])
                nc.sync.dma_start(idst_sb[:], io["idst"][:])

                # ---- Phase B: node tables A, B -> DRAM (bf16) ----
                nch = c.node_chunk
                spg = nch // 128
                with (
                    tc.tile_pool(name="xb", bufs=4) as xpool,
                    tc.tile_pool(name="tst", bufs=3) as spool,
                    tc.tile_pool(name="ps", bufs=6, space="PSUM") as pspool,
                ):
                    for xt, wb, tab, is_a in (
                        (io["xt_dis"], w1at_b, a_tab, True),
                        (io["xt_drug"], w1bt_b, b_tab, False),
                    ):
                        for ci in range(_cdiv(c.n_nodes, nch)):
                            c0 = ci * nch
                            cw = min(nch, c.n_nodes - c0)
                            xb = xpool.tile([128, nch], BF16, tag="xb")
                            nc.sync.dma_start(xb[:, :cw], xt[:, c0:c0 + cw])
                            st = spool.tile([128, spg, 256], BF16, tag="st")
                            full_g = cw // 128
                            rem = cw % 128
                            for g in range(_cdiv(cw, 128)):
                                sw = min(128, cw - g * 128)
                                ps = pspool.tile([128, 256], F32, tag="ps")
                                nc.tensor.matmul(
                                    out=ps[:sw, :],
                                    lhsT=xb[:, g * 128:g * 128 + sw],
                                    rhs=wb[:],
                                    start=True, stop=True,
                                )
                                if is_a:
                                    nc.vector.tensor_add(
                                        st[:sw, g, :], ps[:sw, :], b1_sb[:sw, :])
                                else:
                                    nc.scalar.copy(st[:sw, g, :], ps[:sw, :])
                            if full_g:
                                nc.sync.dma_start(
                                    tab[c0:c0 + full_g * 128, :].rearrange(
                                        "(g p) h -> p g h", p=128),
                                    st[:, :full_g, :],
                                )
                            if rem:
                                nc.sync.dma_start(
                                    tab[c0 + full_g * 128:
                                        c0 + full_g * 128 + rem, :],
                                    st[:rem, full_g, :],
                                )

                # ---- Phase C ----
                wic = c.gi // 16
                with (
                    tc.tile_pool(name="ga", bufs=2) as gapool,
                    tc.tile_pool(name="gb", bufs=2) as gbpool,
                    tc.tile_pool(name="o", bufs=2) as opool,
                ):
                    for b in range(c.nblk):
                        gA = gapool.tile([128, c.ngrp, 256], BF16, tag="gA")
                        gB = gbpool.tile([128, c.ngrp, 256], BF16, tag="gB")
                        for j in range(c.gpb):
                            col0 = (b * c.gpb + j) * wic
                            nc.gpsimd.dma_gather(
                                gA[:, 8 * j:8 * j + 8, :],
                                a_tab[:, :],
                                isrc_sb[:, col0:col0 + wic],
                                c.gi, c.gi, 256,
                                queue_num=j % NQ,
                            )
                            nc.gpsimd.dma_gather(
                                gB[:, 8 * j:8 * j + 8, :],
                                b_tab[:, :],
                                idst_sb[:, col0:col0 + wic],
                                c.gi, c.gi, 256,
                                queue_num=j % NQ,
                            )
                        # h = relu(gA + gB): flat packed add on DVE, relu ACT
                        gAf = gA.rearrange("p g h -> p (g h)")
                        nc.vector.tensor_add(
                            gAf[:], gAf[:], gB.rearrange("p g h -> p (g h)")[:])
                        nc.scalar.activation(
                            gAf[:], gAf[:], mybir.ActivationFunctionType.Relu)
                        # signed tree: t1[0:q] = L - R; t1[q:] = +-(L + R)
                        t1 = gbpool.tile([128, c.ngrp, 128], BF16, tag="t1")
                        if c.q > 0:
                            nc.vector.tensor_sub(
                                t1[:, :, 0:c.q],
                                gA[:, :, 0:c.q], gA[:, :, 128:128 + c.q])
                        if c.q < 128:
                            if c.tail_neg:
                                nc.vector.scalar_tensor_tensor(
                                    out=t1[:, :, c.q:128],
                                    in0=gA[:, :, c.q:128],
                                    scalar=-1.0,
                                    in1=gA[:, :, 128 + c.q:256],
                                    op0=mybir.AluOpType.mult,
                                    op1=mybir.AluOpType.subtract,
                                )
                            else:
                                nc.vector.tensor_add(
                                    t1[:, :, c.q:128],
                                    gA[:, :, c.q:128], gA[:, :, 128 + c.q:256])
                        t2 = gbpool.tile([128, c.ngrp, 64], BF16, tag="t2")
                        nc.vector.tensor_add(
                            t2[:], t1[:, :, 0:64], t1[:, :, 64:128])
                        r = opool.tile([128, c.ngrp], F32, tag="r")
                        nc.vector.tensor_reduce(
                            out=r[:],
                            in_=t2[:],
                            axis=mybir.AxisListType.X,
                            op=mybir.AluOpType.add,
                        )
                        rto = opool.tile([c.ngrp, 128], F32, tag="rto")
                        for bi in range(4):
                            for bj in range(c.ngrp // 32):
                                nc.vector.transpose(
                                    rto[bj * 32:bj * 32 + 32,
                                        bi * 32:bi * 32 + 32],
                                    r[bi * 32:bi * 32 + 32,
                                      bj * 32:bj * 32 + 32],
                                )
                        nc.sync.dma_start(io["out"][b, :, :], rto[:, :])


# ---------------------------------------------------------------------------
# Host side
# ---------------------------------------------------------------------------

_CACHE = {}
last_result = None  # BassKernelResults of the most recent run


def _declare(nc, name, shape, dtype, is_out=False):
    return nc.declare_dram_parameter(name, list(shape), dtype, isOutput=is_out)


def _make_nc(cfg):
    nc = bacc.Bacc("TRN2", target_bir_lowering=False, debug=False,
                   num_devices=N_CORES, num_swdge_queues=NQ,
                   detect_race_conditions=False)
    io = {
        "xt_dis": _declare(nc, "xt_dis", [128, cfg.n_nodes], BF16),
        "xt_drug": _declare(nc, "xt_drug", [128, cfg.n_nodes], BF16),
        "w1at": _declare(nc, "w1at", [128, 256], F32),
        "w1bt": _declare(nc, "w1bt", [128, 256], F32),
        "b1bc": _declare(nc, "b1bc", [128, 256], F32),
        "isrc": _declare(nc, "isrc", [128, cfg.idx_cols], I16),
        "idst": _declare(nc, "idst", [128, cfg.idx_cols], I16),
        "out": _declare(nc, "out", [cfg.nblk, cfg.ngrp, 128], F32,
                        is_out=True),
    }
    build(nc, io, cfg)
    nc.compile()
    return nc


def _get_nc_cached(cfg):
    key = (cfg.n_nodes, cfg.e_core, cfg.q, cfg.tail_neg)
    if key not in _CACHE:
        _CACHE[key] = _make_nc(cfg)
    return _CACHE[key]


def _install_ntff_hook():
    """Shim antenv.axon_hooks (absent in this image) so trace=True works."""
    import antenv
    if "antenv.axon_hooks" in sys.modules:
        return
    m = types.ModuleType("antenv.axon_hooks")
    m._hook = None
    m.set_axon_ntff_profile_hook = lambda h: setattr(m, "_hook", h)
    m.get_axon_ntff_profile_hook = lambda: m._hook
    sys.modules["antenv.axon_hooks"] = m
    antenv.axon_hooks = m
    try:
        from trn_agent_boot.trn_boot import _ntff_profile_via_ctypes
        m.set_axon_ntff_profile_hook(
            _ntff_profile_via_ctypes("/opt/axon/libaxon_pjrt.so"))
    except Exception:
        pass


def wrap_idx(idx_padded, cfg):
    """[e_pad] int16 -> [128, idx_cols] wrapped (16-row pattern x8)."""
    w = idx_padded.reshape(-1, 16).T  # logical i at [i%16, i//16]
    return np.ascontiguousarray(np.tile(w, (8, 1)))


def sign_perm(w2):
    """Hidden permutation [L(128) | R(128)] with cross (pos, neg) pairs
    first; returns (perm[256], q, tail_neg)."""
    sgn = np.sign(w2)
    pos = np.flatnonzero(sgn > 0)
    neg = np.flatnonzero(sgn <= 0)
    m = len(pos)
    q = min(m, 256 - m)
    L = list(pos[:q])
    R = list(neg[:q])
    rest = list(pos[q:]) + list(neg[q:])  # one of these is empty
    tail_neg = m < 128
    n_tail = 128 - q
    L += rest[:n_tail]
    R += rest[n_tail:]
    assert len(L) == len(R) == 128
    return np.array(L + R), q, tail_neg


def prep_in_maps(cfg, perm, x_disease, x_drug, edge_label_index,
                 W1, b1, W2, b2, n_cores=N_CORES):
    w2 = np.asarray(W2, np.float64).reshape(-1)
    absw = np.abs(w2)
    W1s = np.asarray(W1, np.float64) * absw[:, None]
    b1s = np.asarray(b1, np.float64) * absw
    W1p = W1s[perm, :]          # permuted hidden rows
    b1p = b1s[perm]

    xt_dis = np.asarray(x_disease).T.astype(np.float32)
    xt_drug = np.asarray(x_drug).T.astype(np.float32)
    import ml_dtypes
    xt_dis = np.ascontiguousarray(xt_dis.astype(ml_dtypes.bfloat16)).view(
        np.uint16).view(ml_dtypes.bfloat16)
    xt_drug = np.ascontiguousarray(xt_drug.astype(ml_dtypes.bfloat16)).view(
        np.uint16).view(ml_dtypes.bfloat16)
    w1at = np.ascontiguousarray(W1p[:, :128].T, dtype=np.float32)
    w1bt = np.ascontiguousarray(W1p[:, 128:].T, dtype=np.float32)
    b1bc = np.ascontiguousarray(
        np.broadcast_to(b1p.reshape(1, 256), (128, 256)), dtype=np.float32)

    e = np.asarray(edge_label_index)
    in_maps = []
    for core in range(n_cores):
        lo = core * cfg.e_core
        src = np.zeros(cfg.e_pad, np.int16)
        dst = np.zeros(cfg.e_pad, np.int16)
        src[:cfg.e_core] = e[0, lo:lo + cfg.e_core].astype(np.int16)
        dst[:cfg.e_core] = e[1, lo:lo + cfg.e_core].astype(np.int16)
        in_maps.append({
            "xt_dis": xt_dis, "xt_drug": xt_drug,
            "w1at": w1at, "w1bt": w1bt, "b1bc": b1bc,
            "isrc": wrap_idx(src, cfg),
            "idst": wrap_idx(dst, cfg),
        })
    return in_maps


def kernel(x_disease, x_drug, edge_label_index, W1, b1, W2, b2, _trace=False):
    global last_result
    from concourse.bass_utils import run_bass_kernel_spmd

    n_nodes = x_disease.shape[0]
    e_core = _cdiv(np.asarray(edge_label_index).shape[1], N_CORES)
    perm, q, tail_neg = sign_perm(np.asarray(W2, np.float64).reshape(-1))
    cfg = Cfg(n_nodes=n_nodes, e_core=e_core, q=q, tail_neg=tail_neg)
    if _trace:
        _install_ntff_hook()
    nc = _get_nc_cached(cfg)
    in_maps = prep_in_maps(cfg, perm, x_disease, x_drug, edge_label_index,
                           W1, b1, W2, b2)
    res = run_bass_kernel_spmd(nc, in_maps, list(range(N_CORES)),
                               trace=_trace)
    last_result = res
    outs = [res.results[cr]["out"].reshape(-1)[:cfg.e_core]
            for cr in range(N_CORES)]
    b2v = float(np.asarray(b2).reshape(-1)[0])
    return (np.concatenate(outs) + b2v).reshape(-1, 1).astype(np.float32)

